# revision 5
# baseline (speedup 1.0000x reference)
"""BatchHardTripletLoss on 8 Trainium2 NeuronCores.

Strategy (batch/row sharding, per the hint): core c owns anchor rows
[512c, 512c+512). All three [4096,128] tensors are passed to every core,
rolled by 512c rows so that core-local row i corresponds to global row
512c+i and the self-match diagonal sits at a *static* column block
(tile n=0, offset 128m) in every core's program (SPMD-friendly).

Per core:
  - PE transposes the three tensors to [128(d), 4096] (f32r, scaled by -2
    on the rhs copies) and squares them for y2 = ||y_j||^2 row sums.
  - Gram tiles m_ij = y2_j - 2 a_i . y_j are accumulated in PSUM:
    one K=128 f32r matmul (-2 a.y) + one K=1 ones-row matmul (+y2_j)
    (+ one K=128 bf16 BIG-identity matmul masking the diagonal for the
    anchor-anchor / anchor-pos matrices).
  - VectorE min-reduces 4-bank PSUM groups; hardest_neg^2 = a2_i + min.
  - distance_pos, sqrt, softplus, and the partial loss sum are computed
    on-chip; each core emits one scalar (sum of its 512 row losses).
Host sums the 8 partials and divides by 4096.
"""

import sys

if "/opt/trn_rl_repo" not in sys.path:
    sys.path.insert(0, "/opt/trn_rl_repo")

from contextlib import ExitStack

import numpy as np

import concourse.bass as bass
import concourse.tile as tile
from concourse import bacc, bass_utils, mybir

F32 = mybir.dt.float32
F32R = mybir.dt.float32r
BF16 = mybir.dt.bfloat16
AF = mybir.ActivationFunctionType
ALU = mybir.AluOpType

B, D, NCORES = 4096, 128, 8
RB = B // NCORES        # 512 rows per core
NCHUNK = B // 128       # 32 row chunks of 128
MT = RB // 128          # 4 m-tiles per core
NGRP = 2                # n groups per matrix (8 n-tiles of 512 / 4 per group)
GRPW = 2048             # group width (4 PSUM banks)
EPS = 1e-12
BIG = 1.0e38

_CACHE: dict = {}


def _build():
    nc = bacc.Bacc("TRN2", target_bir_lowering=False, debug=False)

    anc = nc.dram_tensor("anc", [B, D], F32, kind="ExternalInput").ap()
    pos = nc.dram_tensor("pos", [B, D], F32, kind="ExternalInput").ap()
    neg = nc.dram_tensor("neg", [B, D], F32, kind="ExternalInput").ap()
    ident = nc.dram_tensor("ident", [D, D], F32, kind="ExternalInput").ap()
    out = nc.dram_tensor("out", [1, 1], F32, kind="ExternalOutput").ap()

    with tile.TileContext(nc) as tc:
        with ExitStack() as ctx:
            _emit(ctx, tc, nc, [anc, pos, neg], ident, out)
    nc.compile()
    return nc


def _emit(ctx, tc, nc, ins, ident_d, out_d):
    const = ctx.enter_context(tc.tile_pool(name="const", bufs=1))
    ytp = ctx.enter_context(tc.tile_pool(name="ytp", bufs=1))
    y2p = ctx.enter_context(tc.tile_pool(name="y2p", bufs=1))
    stats = ctx.enter_context(tc.tile_pool(name="stats", bufs=1))

    # ---- constants ----
    ident = const.tile([128, 128], F32, tag="ident")
    nc.sync.dma_start(ident[:], ident_d)
    eye_big = const.tile([128, 128], BF16, tag="eye_big")
    nc.scalar.activation(eye_big[:], ident[:], AF.Copy, scale=BIG)
    ibuf = const.tile([128, 1024], BF16, tag="ibuf")
    nc.vector.memset(ibuf[:, 0:512], 0.0)
    nc.vector.memset(ibuf[:, 640:1024], 0.0)
    nc.scalar.activation(ibuf[:, 512:640], ident[:], AF.Copy)
    ones_col_f = const.tile([128, 1], F32, tag="ones_col_f")
    nc.vector.memset(ones_col_f[:], 1.0)
    ones128_f = const.tile([128, 128], F32, tag="ones128_f")
    nc.vector.memset(ones128_f[:], 1.0)
    ones128 = const.tile([128, 128], F32R, tag="ones128")
    nc.scalar.activation(ones128[:], ones128_f[:], AF.Copy)

    # ---- persistent transposed operands ----
    # yt[y]: Y^T [128(d), 4096] f32r (rhs of the Gram matmuls)
    # ytsq[y]: 0.5 * (Y^T)^2 -- rhs of the y2-fold matmuls (ones128 @ ytsq
    #   accumulates y2_j/2 onto every partition of the Gram tile)
    # PSUM then holds m'_ij = y2_j/2 - a_i.y_j ; hardest^2 = a2_i + 2*min m'.
    yt = [ytp.tile([128, B], F32R, tag=f"yt{y}", name=f"yt{y}") for y in range(3)]
    ytsq = [y2p.tile([128, B], F32R, tag=f"ytsq{y}", name=f"ytsq{y}")
            for y in range(3)]
    # -A^T for the stationary side (core's own 512 rows)
    at2 = ytp.tile([128, RB], F32R, tag="at2")

    # per-m row stats [128, 4]
    a2col = stats.tile([128, MT], F32, tag="a2col")
    dpsq = stats.tile([128, MT], F32, tag="dpsq")

    # ================= phase A: load, transpose, square =================
    SQS = float(np.sqrt(0.5))
    with ExitStack() as pa:
        raw = pa.enter_context(tc.tile_pool(name="raw", bufs=1))
        scr = pa.enter_context(tc.tile_pool(name="scr", bufs=4))
        tpsum = pa.enter_context(tc.tile_pool(name="tpsum", bufs=4, space="PSUM"))

        raws = []
        for y in range(3):
            r = raw.tile([128, NCHUNK, 128], F32, tag=f"raw{y}", name=f"raw{y}")
            nc.sync.dma_start(r[:], ins[y].rearrange("(n p) d -> p n d", p=128))
            raws.append(r)

        for y in range(3):
            for g in range(8):
                pt = tpsum.tile([128, 512], F32)
                for k in range(4):
                    nc.tensor.transpose(
                        pt[:, k * 128:(k + 1) * 128], raws[y][:, 4 * g + k, :],
                        ident[:],
                    )
                sl = slice(512 * g, 512 * (g + 1))
                # DVE: plain copy Y^T ; ACT: 0.5*(Y^T)^2 via Square(x*sqrt(.5))
                nc.vector.tensor_copy(yt[y][:, sl], pt[:])
                nc.scalar.activation(ytsq[y][:, sl], pt[:], AF.Square, scale=SQS)
                if y == 0 and g == 0:
                    nc.scalar.activation(at2[:], pt[:], AF.Copy, scale=-1.0)

        # row stats for the core's own 512 rows (chunks 0..3 of anc/pos)
        for m in range(MT):
            asq = scr.tile([128, 128], F32, tag="asq")
            nc.scalar.activation(asq[:], raws[0][:, m, :], AF.Square)
            nc.vector.tensor_reduce(out=a2col[:, m:m + 1], in_=asq[:],
                                    axis=mybir.AxisListType.X, op=ALU.add)
            dif = scr.tile([128, 128], F32, tag="dif")
            nc.vector.tensor_tensor(out=dif[:], in0=raws[0][:, m, :],
                                    in1=raws[1][:, m, :], op=ALU.subtract)
            dsq = scr.tile([128, 128], F32, tag="dsq")
            nc.scalar.activation(dsq[:], dif[:], AF.Square)
            nc.vector.tensor_reduce(out=dpsq[:, m:m + 1], in_=dsq[:],
                                    axis=mybir.AxisListType.X, op=ALU.add)

    # ================= phase B: Gram + min-reduce =================
    mins = stats.tile([128, MT, 6], F32, tag="mins")
    with ExitStack() as pb:
        mpsum = pb.enter_context(tc.tile_pool(name="mpsum", bufs=2, space="PSUM"))
        for m in range(MT):
            lhs = at2[:, m * 128:(m + 1) * 128]
            for y in range(3):
                for g in range(NGRP):
                    pg = mpsum.tile([128, GRPW], F32)
                    masked = (g == 0 and y < 2)
                    for k in range(4):
                        n = 4 * g + k
                        nc.tensor.matmul(
                            pg[:, k * 512:(k + 1) * 512], lhs,
                            yt[y][:, n * 512:(n + 1) * 512],
                            start=True, stop=False,
                        )
                    for k in range(4):
                        n = 4 * g + k
                        last = not (masked and k == 0)
                        nc.tensor.matmul(
                            pg[:, k * 512:(k + 1) * 512], ones128[:],
                            ytsq[y][:, n * 512:(n + 1) * 512],
                            start=False, stop=last,
                        )
                    if masked:
                        nc.tensor.matmul(
                            pg[:, 0:512], eye_big[:],
                            ibuf[:, 512 - 128 * m:1024 - 128 * m],
                            start=False, stop=True,
                        )
                    nc.vector.tensor_reduce(
                        out=mins[:, m, 3 * g + y:3 * g + y + 1], in_=pg[:],
                        axis=mybir.AxisListType.X, op=ALU.min,
                    )

    # ================= final: loss =================
    fin = ctx.enter_context(tc.tile_pool(name="fin", bufs=1))
    hnmin = fin.tile([128, MT], F32, tag="hnmin")
    for m in range(MT):
        nc.vector.tensor_reduce(out=hnmin[:, m:m + 1], in_=mins[:, m, :],
                                axis=mybir.AxisListType.X, op=ALU.min)
    hnsq = fin.tile([128, MT], F32, tag="hnsq")
    nc.vector.tensor_scalar(out=hnsq[:], in0=hnmin[:], scalar1=2.0,
                            scalar2=None, op0=ALU.mult)
    nc.vector.tensor_tensor(out=hnsq[:], in0=hnsq[:], in1=a2col[:], op=ALU.add)
    nc.vector.tensor_scalar_max(out=hnsq[:], in0=hnsq[:], scalar1=EPS)
    # sqrt(x) = exp(0.5*ln(x)) -- keeps the whole kernel on one ACT table
    hn = fin.tile([128, MT], F32, tag="hn")
    nc.scalar.activation(hn[:], hnsq[:], AF.Ln)
    nc.scalar.activation(hn[:], hn[:], AF.Exp, scale=0.5)
    nc.vector.tensor_scalar_max(out=dpsq[:], in0=dpsq[:], scalar1=EPS)
    dp = fin.tile([128, MT], F32, tag="dp")
    nc.scalar.activation(dp[:], dpsq[:], AF.Ln)
    nc.scalar.activation(dp[:], dp[:], AF.Exp, scale=0.5)
    x = fin.tile([128, MT], F32, tag="x")
    nc.vector.tensor_tensor(out=x[:], in0=dp[:], in1=hn[:], op=ALU.subtract)
    ex = fin.tile([128, MT], F32, tag="ex")
    nc.scalar.activation(ex[:], x[:], AF.Exp)
    sp = fin.tile([128, MT], F32, tag="sp")
    # softplus(x) = ln(exp(x) + 1); x <= ~20 here so exp cannot overflow
    nc.scalar.activation(sp[:], ex[:], AF.Ln, bias=ones_col_f[:], scale=1.0)
    lsum = fin.tile([128, 1], F32, tag="lsum")
    nc.vector.tensor_reduce(out=lsum[:], in_=sp[:],
                            axis=mybir.AxisListType.X, op=ALU.add)
    with ExitStack() as pf:
        fpsum = pf.enter_context(tc.tile_pool(name="fpsum", bufs=1, space="PSUM"))
        ps = fpsum.tile([1, 1], F32)
        nc.tensor.matmul(ps[:], lsum[:], ones_col_f[:], start=True, stop=True)
        res = fin.tile([1, 1], F32, tag="res")
        nc.scalar.activation(res[:], ps[:], AF.Copy)
    nc.sync.dma_start(out_d, res[:])


def _get_nc():
    if "nc" not in _CACHE:
        _CACHE["nc"] = _build()
    return _CACHE["nc"]


def kernel(rep_anchor, rep_pos, rep_neg):
    A = np.ascontiguousarray(rep_anchor, dtype=np.float32)
    P = np.ascontiguousarray(rep_pos, dtype=np.float32)
    N = np.ascontiguousarray(rep_neg, dtype=np.float32)
    ident = np.eye(D, dtype=np.float32)

    nc = _get_nc()
    in_maps = []
    for c in range(NCORES):
        s = RB * c
        in_maps.append({
            "anc": np.ascontiguousarray(np.concatenate([A[s:], A[:s]], axis=0)),
            "pos": np.ascontiguousarray(np.concatenate([P[s:], P[:s]], axis=0)),
            "neg": np.ascontiguousarray(np.concatenate([N[s:], N[:s]], axis=0)),
            "ident": ident,
        })
    res = bass_utils.run_bass_kernel_spmd(nc, in_maps,
                                          core_ids=list(range(NCORES)))
    total = np.float64(0.0)
    for c in range(NCORES):
        total += np.float64(res.results[c]["out"][0, 0])
    return np.float32(total / B)
